# revision 1
# baseline (speedup 1.0000x reference)
"""Cached grouped-query multi-head attention on 8 Trainium2 cores.

Sharding: core c -> batch b = c//2, head-half = c%2 (8 of 16 heads, 2 of 4
KV groups per core). Wq/Wk column-parallel, Wo row-parallel; the two
partial Wo products per batch are summed on the host (the "all-reduce").

Device kernel (per core, fp32 data / float32r matmuls):
  x^T via PE transposes -> Q^T/K^T/V projections -> RoPE (head-dim stored
  even-dims-then-odd-dims so rotation halves are partition-contiguous;
  host permutes Wq/Wk columns accordingly) -> scores computed transposed
  [k, q] so softmax probs are already PV-ready -> exp (no max subtraction;
  scores are O(1)) -> multiplicative mask on partial tiles only ->
  PV (out^T layout) + all-ones matmul for the softmax denominator ->
  reciprocal scale -> row-parallel Wo -> partial [LQ, D] output.
"""

import math
import sys

import numpy as np

sys.path.insert(0, "/opt/trn_rl_repo")

B, LQ, D = 4, 1024, 2048
H, G = 16, 4
HD = 128            # head dim
GS = H // G         # heads per group
PAST = 1024
LK = PAST + LQ      # 2048
NCORES = 8
NH = 8              # local heads per core
NG = 2              # local groups per core
KSUB = D // 128     # 16 contraction subtiles over D
QC = LQ // 512      # 2 query chunks of 512
QS = LQ // 128      # 8 query subtiles of 128
KC = LK // 128      # 16 key chunks of 128
NCH = D // 512      # 4 output column chunks

_PERM = np.concatenate([np.arange(0, HD, 2), np.arange(1, HD, 2)])
_PROG_CACHE = {}


ATTN_BF16 = True  # bf16 scores/PV/den (2x LDW pipelining, ~3e-3 rel err)


def _build_program(classes, n_part, attn_bf16=False):
    """Build the per-core Bass/Tile program.

    classes[(qc, kc)] = ("full"|"skip"|"part", partial_idx_or_None),
    identical on every core (the mask is shared).
    """
    import concourse.bacc as bacc
    import concourse.mybir as mybir
    import concourse.tile as tile
    from concourse.masks import make_identity

    f32 = mybir.dt.float32
    f32r = mybir.dt.float32r
    adt = mybir.dt.bfloat16 if attn_bf16 else f32r
    AF = mybir.ActivationFunctionType
    OP = mybir.AluOpType

    nc = bacc.Bacc("TRN2", target_bir_lowering=False, debug=False,
                   num_devices=NCORES)

    x_d = nc.dram_tensor("x", [LQ, D], f32, kind="ExternalInput").ap()
    wq_d = nc.dram_tensor("wq", [D, NH * HD], f32r, kind="ExternalInput").ap()
    bq_d = nc.dram_tensor("bq", [NH, HD, 1], f32, kind="ExternalInput").ap()
    wk_d = nc.dram_tensor("wk", [D, NG * HD], f32r, kind="ExternalInput").ap()
    bk_d = nc.dram_tensor("bk", [NG, HD, 1], f32, kind="ExternalInput").ap()
    wv_d = nc.dram_tensor("wv", [D, NG * HD], f32r, kind="ExternalInput").ap()
    bv_d = nc.dram_tensor("bv", [1, NG * HD], f32, kind="ExternalInput").ap()
    pk_d = nc.dram_tensor("pk", [NG, PAST, HD], f32, kind="ExternalInput").ap()
    pv_d = nc.dram_tensor("pv", [NG, PAST, HD], adt, kind="ExternalInput").ap()
    rot_d = nc.dram_tensor("rot", [LQ, HD // 2], f32, kind="ExternalInput").ap()
    wo_d = nc.dram_tensor("wo", [NH * HD, D], f32r, kind="ExternalInput").ap()
    bo_d = nc.dram_tensor("bo", [1, D], f32, kind="ExternalInput").ap()
    mp_d = None
    if n_part:
        mp_d = nc.dram_tensor("maskp", [n_part, 128, 512], adt,
                              kind="ExternalInput").ap()
    out_d = nc.dram_tensor("out", [LQ, D], f32, kind="ExternalOutput").ap()

    # active key chunks per query chunk: list of (kc, partial_idx|None)
    active = {qc: [(kc, classes[(qc, kc)][1])
                   for kc in range(KC) if classes[(qc, kc)][0] != "skip"]
              for qc in range(QC)}

    scl = 1.0 / math.sqrt(HD)

    with tile.TileContext(nc) as tc:
        with (
            tc.tile_pool(name="const", bufs=1) as const,
            tc.tile_pool(name="persist", bufs=1) as persist,
            tc.tile_pool(name="raw", bufs=2) as raw,
            tc.tile_pool(name="ropet", bufs=1) as ropetp,
        ):
            ident = const.tile([128, 128], f32)
            make_identity(nc, ident)
            ones_f = const.tile([128, 128], f32)
            nc.gpsimd.memset(ones_f, 1.0)
            ones_mat = const.tile([128, 128], adt)
            nc.vector.tensor_copy(ones_mat, ones_f)

            bias_qk = const.tile([128, NH + NG], f32)
            for h in range(NH):
                nc.sync.dma_start(bias_qk[:, h:h + 1], bq_d[h])
            for g in range(NG):
                nc.sync.dma_start(bias_qk[:, NH + g:NH + g + 1], bk_d[g])

            QT = persist.tile([128, NH, LQ], adt)     # roped Q^T (perm rows)
            KT = persist.tile([128, NG, LK], adt)     # K^T cache (perm rows)
            V = [persist.tile([128, KC, HD], adt, tag=f"v{g}", name=f"v{g}")
                 for g in range(NG)]

            # full-height rotary tables: rows 0:64 and 64:128 both hold the
            # 64 frequencies; ssgnF carries -sin on top, +sin on bottom, so
            #   roped = src*cosF + swap(src)*ssgnF
            # where swap exchanges the two partition halves (x1<->x2):
            #   top: x1*cos + x2*(-sin)   bot: x2*cos + x1*(+sin)
            cosF = const.tile([128, LQ], f32)
            ssgnF = const.tile([128, LQ], f32)

            def rope(src, dst):
                # src/dst [128, LQ]; rows 0:64 = even dims, 64:128 = odd
                swp = raw.tile([128, LQ], f32, tag="raw", name="swp")
                nc.sync.dma_start(swp[0:64], src[64:128])
                nc.sync.dma_start(swp[64:128], src[0:64])
                t = ropetp.tile([128, LQ], f32, tag="ropet")
                nc.vector.tensor_mul(t, swp, ssgnF)
                nc.vector.tensor_mul(dst, src, cosF)
                nc.vector.tensor_tensor(dst, dst, t, OP.add)

            # ---- phase 1: rotary tables + x^T ----
            with (
                tc.tile_pool(name="xt", bufs=1) as xtp,
                tc.tile_pool(name="pstp", bufs=3, space="PSUM") as pstp,
            ):
                xT = xtp.tile([128, KSUB, LQ], f32r)
                # K/V weight tiles; DMAs are issued after the x/rot loads
                # so they don't delay the critical-path x^T build (LIFO:
                # wvp closes after the V projection, wkp after K)
                wk_cm = tc.tile_pool(name="wkp", bufs=2)
                wkp = wk_cm.__enter__()
                wkgs = [wkp.tile([128, KSUB, HD], f32r, tag="wk",
                                 name=f"wk{g}") for g in range(NG)]
                wv_cm = tc.tile_pool(name="wvp", bufs=1)
                wvp = wv_cm.__enter__()
                wvt = wvp.tile([128, KSUB, NG * HD], f32r)

                def load_kv_weights():
                    for g in range(NG):
                        nc.sync.dma_start(
                            wkgs[g],
                            wk_d.rearrange("(ko ki) m -> ki ko m", ki=128)
                            [:, :, g * HD:(g + 1) * HD])
                    nc.sync.dma_start(
                        wvt, wv_d.rearrange("(ko ki) m -> ki ko m", ki=128))
                with tc.tile_pool(name="ph1", bufs=1) as ph1:
                    # ssgnF[:, :512] and cosF[0:64] double as scratch for
                    # the rotary load/transpose; both are overwritten after
                    rall = ssgnF.rearrange("p (i f) -> p i f", f=64)
                    nc.sync.dma_start(
                        rall[:, 0:8, :],
                        rot_d.rearrange("(i p) f -> p i f", p=128))
                    rotT = cosF[0:64]
                    for i in range(8):
                        ps = pstp.tile([128, 128], f32, tag="tp")
                        nc.tensor.transpose(ps[0:64, :], rall[:, i, :], ident)
                        nc.vector.tensor_copy(rotT[:, i * 128:(i + 1) * 128],
                                              ps[0:64, :])
                    # freq in [0, 2pi); Sin on ScalarE needs [-pi, pi]:
                    #   -sin(x) = sin(x - pi);  cos(x) = 1 - 2*sin^2(x/2)
                    negpi = const.tile([64, 1], f32)
                    nc.gpsimd.memset(negpi, -math.pi)
                    nc.scalar.activation(ssgnF[0:64], rotT, AF.Sin,
                                         bias=negpi)
                    s2 = ropetp.tile([64, LQ], f32, tag="ropet",
                                     name="s2")
                    nc.scalar.activation(s2, rotT, AF.Sin, scale=0.5)
                    nc.vector.tensor_mul(s2, s2, s2)
                    nc.vector.tensor_scalar(cosF[0:64], s2, -2.0, 1.0,
                                            OP.mult, OP.add)
                    # replicate to the bottom half (sin with flipped sign)
                    nc.vector.tensor_scalar_mul(s2, ssgnF[0:64], -1.0)
                    nc.sync.dma_start(ssgnF[64:128], s2)
                    nc.sync.dma_start(cosF[64:128], cosF[0:64])

                    for i in range(QS):
                        for quart in range(4):
                            xc = ph1.tile([128, 512], f32, tag="xc",
                                          bufs=3)
                            nc.sync.dma_start(
                                xc, x_d[i * 128:(i + 1) * 128,
                                        quart * 512:(quart + 1) * 512])
                            for jj in range(4):
                                j = quart * 4 + jj
                                ps = pstp.tile([128, 128], f32, tag="tp")
                                nc.tensor.transpose(
                                    ps, xc[:, jj * 128:(jj + 1) * 128], ident)
                                nc.vector.tensor_copy(
                                    xT[:, j, i * 128:(i + 1) * 128], ps)
                            if i == 1 and quart == 3:
                                load_kv_weights()

                # ---- phase 2: projections (V, K + past KV, then Q) ----
                with tc.tile_pool(name="psproj", bufs=2,
                                  space="PSUM") as psproj:
                    # V = x @ Wv + bv  (natural [q, dv] layout)
                    if True:
                        bv_sb = const.tile([1, NG * HD], f32)
                        nc.sync.dma_start(bv_sb, bv_d)
                        bv_rep = const.tile([128, NG * HD], f32)
                        nc.gpsimd.partition_broadcast(bv_rep, bv_sb)
                        for qs in range(QS):
                            ps = psproj.tile([128, 512], f32)
                            for ko in range(KSUB):
                                nc.tensor.matmul(
                                    ps[:, :NG * HD],
                                    xT[:, ko,
                                       qs * 128:(qs + 1) * 128],
                                    wvt[:, ko, :],
                                    start=(ko == 0), stop=(ko == KSUB - 1))
                            for g in range(NG):
                                nc.vector.tensor_tensor(
                                    V[g][:, PAST // 128 + qs, :],
                                    ps[:, g * HD:(g + 1) * HD],
                                    bv_rep[:, g * HD:(g + 1) * HD], OP.add)

                    wv_cm.__exit__(None, None, None)
                    # K^T (roped) + past K^T (permuted transpose) + past V
                    if True:
                        for g in range(NG):
                            wkg = wkgs[g]
                            bkt = bias_qk[:, NH + g:NH + g + 1]
                            kraw = raw.tile([128, LQ], f32, tag="raw")
                            for qc in range(QC):
                                ps = psproj.tile([128, 512], f32)
                                for ko in range(KSUB):
                                    nc.tensor.matmul(
                                        ps,
                                        wkg[:, ko, :],
                                        xT[:, ko, qc * 512:(qc + 1) * 512]
                                        ,
                                        start=(ko == 0),
                                        stop=(ko == KSUB - 1))
                                nc.vector.tensor_scalar_add(
                                    kraw[:, qc * 512:(qc + 1) * 512], ps, bkt)
                            rope(kraw, KT[:, g, PAST:])

                            # pk head-dim is pre-permuted on the host, so a
                            # plain transpose lands rows in rope layout
                            for kc in range(PAST // 128):
                                pkc = raw.tile([128, HD], f32, tag="pkc")
                                nc.sync.dma_start(
                                    pkc, pk_d[g, kc * 128:(kc + 1) * 128, :])
                                ps = pstp.tile([128, 128], f32, tag="tp")
                                nc.tensor.transpose(ps, pkc, ident)
                                nc.vector.tensor_copy(
                                    KT[:, g, kc * 128:(kc + 1) * 128], ps)
                                nc.sync.dma_start(
                                    V[g][:, kc, :],
                                    pv_d[g, kc * 128:(kc + 1) * 128, :])

                    wk_cm.__exit__(None, None, None)
                    # Q^T (roped), per head
                    with tc.tile_pool(name="wqp", bufs=2) as wqp:
                        for h in range(NH):
                            wqh = wqp.tile([128, KSUB, HD], f32r, tag="wq")
                            nc.sync.dma_start(
                                wqh, wq_d.rearrange("(ko ki) m -> ki ko m",
                                                    ki=128)
                                [:, :, h * HD:(h + 1) * HD])
                            bqt = bias_qk[:, h:h + 1]
                            qraw = raw.tile([128, LQ], f32, tag="raw")
                            for qc in range(QC):
                                ps = psproj.tile([128, 512], f32)
                                for ko in range(KSUB):
                                    nc.tensor.matmul(
                                        ps,
                                        wqh[:, ko, :],
                                        xT[:, ko, qc * 512:(qc + 1) * 512]
                                        ,
                                        start=(ko == 0),
                                        stop=(ko == KSUB - 1))
                                nc.vector.tensor_scalar_add(
                                    qraw[:, qc * 512:(qc + 1) * 512], ps, bqt)
                            rope(qraw, QT[:, h, :])

            # ---- phase 4: attention ----
            import contextlib
            ph45 = contextlib.ExitStack()
            attnp = ph45.enter_context(tc.tile_pool(name="attnp", bufs=1))
            attnT = attnp.tile([128, NH, LQ], f32r)
            wop = ph45.enter_context(tc.tile_pool(name="wop", bufs=2))

            def load_wot(ncH):
                wot = wop.tile([128, NH, 512], f32r, tag="wo",
                               name=f"wo{ncH}")
                nc.sync.dma_start(
                    wot, wo_d.rearrange("(ho hi) n -> hi ho n", hi=128)
                    [:, :, ncH * 512:(ncH + 1) * 512])
                return wot

            wot0 = load_wot(0)
            with (
                tc.tile_pool(name="mpp", bufs=1) as mpp,
                tc.tile_pool(name="ptp", bufs=4) as ptp,
                tc.tile_pool(name="pssc", bufs=3, space="PSUM") as pssc,
                tc.tile_pool(name="pspv", bufs=3, space="PSUM") as pspv,
                tc.tile_pool(name="psdn", bufs=2, space="PSUM") as psdn,
            ):
                mp_sb = None
                if n_part:
                    mp_sb = mpp.tile([128, n_part, 512], adt)
                    for i in range(n_part):
                        nc.sync.dma_start(mp_sb[:, i, :], mp_d[i])

                for h in range(NH):
                    g = h // GS
                    for qc in range(QC):
                        act = active[qc]
                        n_act = len(act)
                        ps_pv = pspv.tile([128, 512], f32)
                        ps_dn = psdn.tile([128, 512], f32)
                        for i, (kc, midx) in enumerate(act):
                            ps_s = pssc.tile([128, 512], f32)
                            nc.tensor.matmul(
                                ps_s,
                                KT[:, g, kc * 128:(kc + 1) * 128]
                                ,
                                QT[:, h, qc * 512:(qc + 1) * 512]
                                ,
                                start=True, stop=True)
                            pt = ptp.tile([128, 512], adt, tag="pt")
                            nc.scalar.activation(pt, ps_s, AF.Exp, scale=scl)
                            if midx is not None:
                                nc.vector.tensor_mul(pt, pt,
                                                     mp_sb[:, midx, :])
                            nc.tensor.matmul(
                                ps_pv, V[g][:, kc, :],
                                pt,
                                start=(i == 0), stop=(i == n_act - 1))
                            nc.tensor.matmul(
                                ps_dn[0:1, :], ones_mat[:, 0:1],
                                pt,
                                start=(i == 0), stop=(i == n_act - 1))
                        rec1 = raw.tile([1, 512], f32, tag="rec1")
                        nc.vector.reciprocal(rec1, ps_dn[0:1, :])
                        rec = raw.tile([128, 512], f32, tag="rec")
                        nc.gpsimd.partition_broadcast(rec, rec1)
                        nc.vector.tensor_mul(
                            attnT[:, h, qc * 512:(qc + 1) * 512], ps_pv, rec)

            # ---- phase 5: output projection ----
            with (
                tc.tile_pool(name="bop", bufs=1) as bop,
                tc.tile_pool(name="pso", bufs=4, space="PSUM") as pso,
            ):
                bo_sb = bop.tile([1, D], f32)
                nc.sync.dma_start(bo_sb, bo_d)
                bo_rep = bop.tile([128, D], f32)
                nc.gpsimd.partition_broadcast(bo_rep, bo_sb)
                for ncH in range(NCH):
                    wot = wot0 if ncH == 0 else load_wot(ncH)
                    for qs in range(QS):
                        ps = pso.tile([128, 512], f32)
                        for h in range(NH):
                            nc.tensor.matmul(
                                ps,
                                attnT[:, h, qs * 128:(qs + 1) * 128]
                                ,
                                wot[:, h, :],
                                start=(h == 0), stop=(h == NH - 1))
                        ot = raw.tile([128, 512], f32, tag="ot")
                        nc.vector.tensor_tensor(
                            ot, ps, bo_rep[:, ncH * 512:(ncH + 1) * 512],
                            OP.add)
                        nc.sync.dma_start(
                            out_d[qs * 128:(qs + 1) * 128,
                                  ncH * 512:(ncH + 1) * 512], ot)
            ph45.close()

    nc.compile()
    return nc


def _classify_mask(mask):
    """Per-[128k x 512q] tile: full / skip / partial (+ fp32 tile data)."""
    mT = mask.T  # [LK, LQ]
    classes = {}
    partials = []
    for qc in range(QC):
        for kc in range(KC):
            t = mT[kc * 128:(kc + 1) * 128, qc * 512:(qc + 1) * 512]
            if t.all():
                classes[(qc, kc)] = ("full", None)
            elif not t.any():
                classes[(qc, kc)] = ("skip", None)
            else:
                classes[(qc, kc)] = ("part", len(partials))
                partials.append(np.ascontiguousarray(t, dtype=np.float32))
    maskp = np.stack(partials) if partials else None
    return classes, maskp


def _prep_in_maps(x, mask, rotary_freqs, past_k, past_v, Wq, bq, Wk, bk,
                  Wv, bv, Wo, bo, maskp, n_part, attn_bf16=False):
    c32 = lambda a: np.ascontiguousarray(a, dtype=np.float32)
    if attn_bf16:
        import ml_dtypes
        cat = lambda a: np.ascontiguousarray(a, dtype=ml_dtypes.bfloat16)
    else:
        cat = c32
    in_maps = []
    for c in range(NCORES):
        b, half = c // 2, c % 2
        h0 = half * NH          # first global head
        g0 = half * NG          # first global group
        wq_c = np.concatenate(
            [Wq[:, (h0 + h) * HD + _PERM] for h in range(NH)], axis=1)
        bq_c = np.stack([bq[(h0 + h) * HD + _PERM] for h in range(NH)])
        wk_c = np.concatenate(
            [Wk[:, (g0 + g) * HD + _PERM] for g in range(NG)], axis=1)
        bk_c = np.stack([bk[(g0 + g) * HD + _PERM] for g in range(NG)])
        m = {
            "x": c32(x[b]),
            "wq": c32(wq_c),
            "bq": c32(bq_c[..., None]),
            "wk": c32(wk_c),
            "bk": c32(bk_c[..., None]),
            "wv": c32(Wv[:, g0 * HD:(g0 + NG) * HD]),
            "bv": c32(bv[g0 * HD:(g0 + NG) * HD][None, :]),
            "pk": c32(past_k[b, g0:g0 + NG][..., _PERM]),
            "pv": cat(past_v[b, g0:g0 + NG]),
            "rot": c32(rotary_freqs),
            "wo": c32(Wo[h0 * HD:(h0 + NH) * HD, :]),
            "bo": c32(bo[None, :] if half == 0 else np.zeros((1, D))),
        }
        if n_part:
            m["maskp"] = cat(maskp)
        in_maps.append(m)
    return in_maps


def _run(inputs, trace=False):
    from concourse import bass_utils

    classes, maskp = _classify_mask(np.asarray(inputs["mask"]))
    n_part = 0 if maskp is None else maskp.shape[0]
    key = (tuple(sorted(classes.items())), ATTN_BF16)
    if key not in _PROG_CACHE:
        _PROG_CACHE[key] = _build_program(classes, n_part,
                                          attn_bf16=ATTN_BF16)
    nc = _PROG_CACHE[key]

    in_maps = _prep_in_maps(
        np.asarray(inputs["x"]), np.asarray(inputs["mask"]),
        np.asarray(inputs["rotary_freqs"]), np.asarray(inputs["past_k"]),
        np.asarray(inputs["past_v"]), np.asarray(inputs["Wq"]),
        np.asarray(inputs["bq"]), np.asarray(inputs["Wk"]),
        np.asarray(inputs["bk"]), np.asarray(inputs["Wv"]),
        np.asarray(inputs["bv"]), np.asarray(inputs["Wo"]),
        np.asarray(inputs["bo"]), maskp, n_part, attn_bf16=ATTN_BF16)

    res = bass_utils.run_bass_kernel_spmd(
        nc, in_maps, list(range(NCORES)), trace=trace,
        trace_cores=list(range(NCORES)) if trace else None)

    out = np.empty((B, LQ, D), np.float32)
    for b in range(B):
        out[b] = res.results[2 * b]["out"] + res.results[2 * b + 1]["out"]
    return out, res


def kernel(**inputs) -> np.ndarray:
    out, _ = _run(inputs, trace=False)
    return out



# revision 5
# speedup vs baseline: 1.0152x; 1.0152x over previous
"""Cached grouped-query multi-head attention on 8 Trainium2 cores.

Sharding: core c -> batch b = c//2, head-half = c%2 (8 of 16 heads, 2 of 4
KV groups per core). Wq/Wk column-parallel, Wo row-parallel; the two
partial Wo products per batch are summed on the host (the "all-reduce"),
which also adds bo.

Host pre-layout (pure relayout, same class as the baseline's weight
permutations): x is shipped pre-transposed as xt[ki, ko, q], past_k
pre-transposed into KT layout, past_v in V-tile layout, rot transposed,
and every weight in a partition-contiguous tile layout, so the device
does ZERO PE transposes and every DMA is large-run.

Device kernel (per core, bf16 attention / f32r projections):
  K proj (interleaved with the chunked xt DMA) -> rope -> KT
  V proj -> V tiles; Q proj -> rope -> QT
  attention per (g, qc, head-pair): scores for 2 heads into a [128,2,512]
  2-bank psum supertile -> ONE exp for both heads (scale folded) ->
  diagonal tiles get shrunk matmuls plus a [128,128] triangular mask ->
  PV + den matmuls (den rows land on psum partitions 0/32 so one
  reciprocal covers both heads) -> normalize into attnT bf16
  Wo: full Wo prefetched bf16 during attention; attnT-stationary loop,
  psum [128,2048] accumulated over heads, scalar-copy + store per 128 q.
"""

import math
import sys

import numpy as np

sys.path.insert(0, "/opt/trn_rl_repo")

B, LQ, D = 4, 1024, 2048
H, G = 16, 4
HD = 128            # head dim
GS = H // G         # heads per group
PAST = 1024
LK = PAST + LQ      # 2048
NCORES = 8
NH = 8              # local heads per core
NG = 2              # local groups per core
KSUB = D // 128     # 16 contraction subtiles over D
QC = LQ // 512      # 2 query chunks of 512
QS = LQ // 128      # 8 query subtiles of 128
KC = LK // 128      # 16 key chunks of 128

_PERM = np.concatenate([np.arange(0, HD, 2), np.arange(1, HD, 2)])
_PROG_CACHE = {}


def _build_program(active):
    """active[qc] = [(kc, dcol, diag)]: dcol = first allowed query column
    (0 for full tiles); diag tiles have a triangular [128,128] block at
    query columns [dcol, dcol+128) and are fully allowed after it."""
    import concourse.bacc as bacc
    import concourse.mybir as mybir
    import concourse.tile as tile

    f32 = mybir.dt.float32
    f32r = mybir.dt.float32r
    bf16 = mybir.dt.bfloat16
    AF = mybir.ActivationFunctionType
    OP = mybir.AluOpType

    nc = bacc.Bacc("TRN2", target_bir_lowering=False, debug=False,
                   num_devices=NCORES)

    xt_d = nc.dram_tensor("xt", [128, KSUB * LQ], f32r,
                          kind="ExternalInput").ap()
    wqa_d = nc.dram_tensor("wqa", [128, KSUB * 512], f32r,
                           kind="ExternalInput").ap()
    wqb_d = nc.dram_tensor("wqb", [128, KSUB * 512], f32r,
                           kind="ExternalInput").ap()
    wk_d = nc.dram_tensor("wk", [128, KSUB * 256], f32r,
                          kind="ExternalInput").ap()
    wv_d = nc.dram_tensor("wv", [128, KSUB * 256], f32r,
                          kind="ExternalInput").ap()
    bqk_d = nc.dram_tensor("bqk", [128, NH + NG], f32,
                           kind="ExternalInput").ap()
    bv_d = nc.dram_tensor("bv", [1, NG * HD], f32, kind="ExternalInput").ap()
    pkt_d = nc.dram_tensor("pkt", [128, NG * PAST], bf16,
                           kind="ExternalInput").ap()
    pv_d = nc.dram_tensor("pv", [128, NG * PAST], bf16,
                          kind="ExternalInput").ap()
    rott_d = nc.dram_tensor("rott", [64, LQ], f32, kind="ExternalInput").ap()
    wo_d = nc.dram_tensor("wo", [128, NH * D], bf16,
                          kind="ExternalInput").ap()
    tri_d = nc.dram_tensor("tri", [128, 256], bf16, kind="ExternalInput").ap()
    out_d = nc.dram_tensor("out", [LQ, D], f32, kind="ExternalOutput").ap()

    scl = 1.0 / math.sqrt(HD)

    with tile.TileContext(nc) as tc:
        with (
            tc.tile_pool(name="const", bufs=1) as const,
            tc.tile_pool(name="persist", bufs=1) as persist,
        ):
            QT = persist.tile([128, NH, LQ], bf16)      # roped Q^T (perm rows)
            KT = persist.tile([128, NG, LK], bf16)      # K^T cache (perm rows)
            V = persist.tile([128, NG, KC, HD], bf16)   # [k, g, kc, hd]
            attnT = persist.tile([128, NH, LQ], bf16)   # normalized attn^T

            # critical-path loads first on the sync HWDGE queue
            with (
                tc.tile_pool(name="xtp", bufs=1) as xtp,
                tc.tile_pool(name="ropec", bufs=1) as ropec,
                tc.tile_pool(name="ropew", bufs=2) as ropew,
            ):
                xt = xtp.tile([128, KSUB, LQ], f32r)
                cosF = ropec.tile([128, LQ], f32)
                ssgnF = ropec.tile([128, LQ], f32)
                rstage = ropec.tile([64, LQ], f32)
                s2 = ropec.tile([64, LQ], f32)

                wkv_cm = tc.tile_pool(name="wkv", bufs=1)
                wkvp = wkv_cm.__enter__()
                wkt = wkvp.tile([128, KSUB, NG * HD], f32r, name="wk")
                wvt = wkvp.tile([128, KSUB, NG * HD], f32r, name="wv")

                # sync HWDGE queue: critical-path loads in consumption order
                nc.sync.dma_start(wkt, wk_d.rearrange("p (ko m) -> p ko m",
                                                      m=NG * HD))
                xt_r = xt_d.rearrange("p (ko q) -> p ko q", q=LQ)
                for j in range(4):
                    nc.sync.dma_start(xt[:, 4 * j:4 * (j + 1), :],
                                      xt_r[:, 4 * j:4 * (j + 1), :])
                nc.sync.dma_start(wvt, wv_d.rearrange("p (ko m) -> p ko m",
                                                      m=NG * HD))

                # secondary loads on the scalar HWDGE queue
                ones_f = const.tile([128, 1], f32)
                nc.gpsimd.memset(ones_f, 1.0)
                ones_c = const.tile([128, 1], bf16)
                nc.vector.tensor_copy(ones_c, ones_f)
                tri2 = const.tile([128, 2, 128], bf16)
                nc.scalar.dma_start(tri2,
                                    tri_d.rearrange("p (i f) -> p i f", f=128))
                bias_qk = const.tile([128, NH + NG], f32)
                nc.scalar.dma_start(bias_qk, bqk_d)
                bv_sb = const.tile([1, NG * HD], f32)
                nc.scalar.dma_start(bv_sb, bv_d)
                bv_rep = const.tile([128, NG * HD], f32)
                nc.gpsimd.partition_broadcast(bv_rep, bv_sb)
                nc.scalar.dma_start(
                    KT[:, :, 0:PAST],
                    pkt_d.rearrange("p (g f) -> p g f", g=NG))
                nc.scalar.dma_start(
                    V[:, :, 0:PAST // 128, :],
                    pv_d.rearrange("p (g kc hd) -> p g kc hd", g=NG, hd=HD))

                # rotary tables: rows 0:64 = even dims, 64:128 = odd;
                # ssgnF = -sin on top, +sin on bottom, so
                # roped = src*cosF + swap(src)*ssgnF
                nc.scalar.dma_start(rstage, rott_d)
                negpi = const.tile([64, 1], f32)
                nc.gpsimd.memset(negpi, -math.pi)
                # -sin(x) = sin(x - pi); cos(x) = 1 - 2*sin^2(x/2)
                nc.scalar.activation(ssgnF[0:64], rstage, AF.Sin,
                                     bias=negpi)
                nc.scalar.activation(s2, rstage, AF.Sin, scale=0.5)
                nc.vector.tensor_mul(s2, s2, s2)
                nc.vector.tensor_scalar(cosF[0:64], s2, -2.0, 1.0,
                                        OP.mult, OP.add)
                nc.vector.tensor_scalar_mul(s2, ssgnF[0:64], -1.0)
                nc.sync.dma_start(ssgnF[64:128], s2)
                nc.sync.dma_start(cosF[64:128], cosF[0:64])

                def rope(src, dst):
                    # src [128, LQ] f32 (clobbered); dst any dtype
                    swp = ropew.tile([128, LQ], f32, tag="swp")
                    nc.sync.dma_start(swp[0:64], src[64:128])
                    nc.sync.dma_start(swp[64:128], src[0:64])
                    t = ropew.tile([128, LQ], f32, tag="ropet")
                    nc.vector.tensor_mul(t, swp, ssgnF)
                    nc.vector.tensor_mul(src, src, cosF)
                    nc.vector.tensor_tensor(dst, src, t, OP.add)

                # ---- K proj, interleaved with the xt DMA chunks ----
                with (
                    tc.tile_pool(name="rawk", bufs=2) as rawk,
                    tc.tile_pool(name="pskp", bufs=1,
                                 space="PSUM") as pskp,
                ):
                    kps = [pskp.tile([128, 512], f32, name=f"kps{i}")
                           for i in range(4)]
                    for j in range(4):
                        for g in range(NG):
                            for qc in range(QC):
                                ps = kps[g * QC + qc]
                                for kk in range(4):
                                    ko = 4 * j + kk
                                    nc.tensor.matmul(
                                        ps,
                                        wkt[:, ko,
                                            g * HD:(g + 1) * HD],
                                        xt[:, ko,
                                           qc * 512:(qc + 1) * 512],
                                        start=(ko == 0),
                                        stop=(ko == KSUB - 1),
                                        skip_group_check=True)
                    kraws = []
                    for g in range(NG):
                        kraw = rawk.tile([128, LQ], f32, tag="kraw",
                                         name=f"kraw{g}")
                        for qc in range(QC):
                            nc.vector.tensor_scalar_add(
                                kraw[:, qc * 512:(qc + 1) * 512],
                                kps[g * QC + qc],
                                bias_qk[:, NH + g:NH + g + 1])
                        kraws.append(kraw)
                    for g in range(NG):
                        rope(kraws[g], KT[:, g, PAST:])

                # ---- V proj ----
                with tc.tile_pool(name="psv", bufs=4,
                                  space="PSUM") as psv:
                    for qs in range(QS):
                        ps = psv.tile([128, NG * HD], f32)
                        for ko in range(KSUB):
                            nc.tensor.matmul(
                                ps,
                                xt[:, ko, qs * 128:(qs + 1) * 128],
                                wvt[:, ko, :],
                                start=(ko == 0),
                                stop=(ko == KSUB - 1))
                        for g in range(NG):
                            nc.vector.tensor_tensor(
                                V[:, g, PAST // 128 + qs, :],
                                ps[:, g * HD:(g + 1) * HD],
                                bv_rep[:, g * HD:(g + 1) * HD],
                                OP.add)

                wkv_cm.__exit__(None, None, None)

                # ---- Q proj ----
                with (
                    tc.tile_pool(name="wqp", bufs=1) as wqp,
                    tc.tile_pool(name="rawq", bufs=2) as rawq,
                    tc.tile_pool(name="psq", bufs=4,
                                 space="PSUM") as psq,
                ):
                    for hh in range(2):
                        wqh = wqp.tile([128, KSUB, 512], f32r,
                                       tag="wq")
                        nc.sync.dma_start(
                            wqh, (wqa_d if hh == 0 else wqb_d)
                            .rearrange("p (ko m) -> p ko m", m=512))
                        for hl in range(4):
                            h = hh * 4 + hl
                            qraw = rawq.tile([128, LQ], f32,
                                             tag="qraw")
                            for qc in range(QC):
                                ps = psq.tile([128, 512], f32)
                                for ko in range(KSUB):
                                    nc.tensor.matmul(
                                        ps,
                                        wqh[:, ko,
                                            hl * 128:(hl + 1) * 128],
                                        xt[:, ko,
                                           qc * 512:(qc + 1) * 512],
                                        start=(ko == 0),
                                        stop=(ko == KSUB - 1))
                                nc.vector.tensor_scalar_add(
                                    qraw[:, qc * 512:(qc + 1) * 512],
                                    ps, bias_qk[:, h:h + 1])
                            rope(qraw, QT[:, h, :])

            # ---- attention ----
            with (
                tc.tile_pool(name="wop", bufs=1) as wop,
                tc.tile_pool(name="ptp", bufs=3) as ptp,
                tc.tile_pool(name="unp", bufs=4) as unp,
                tc.tile_pool(name="recp", bufs=2) as recp,
            ):
                # prefetch full Wo (bf16) for phase 5
                wot = wop.tile([128, NH, D], bf16)
                nc.sync.dma_start(
                    wot, wo_d.rearrange("p (h n) -> p h n", n=D))

                with (
                    tc.tile_pool(name="psst", bufs=2, space="PSUM") as psst,
                    tc.tile_pool(name="pspv", bufs=2, space="PSUM") as pspv,
                    tc.tile_pool(name="psdn", bufs=2, space="PSUM") as psdn,
                ):
                    for g in range(NG):
                        for qc in range(QC):
                            act = active[qc]
                            kc0, kcL = act[0][0], act[-1][0]
                            for half in range(2):
                                h0 = g * GS + half * 2
                                ps_pv = [
                                    pspv.tile([128, 512], f32, tag="pv",
                                              name=f"pv{half}{hi}")
                                    for hi in range(2)]
                                ps_dn = psdn.tile([128, 512], f32, tag="dn")
                                nc.vector.memset(ps_dn, 1.0)
                                prev = None

                                def pv_den(item):
                                    kc, dcol, pt = item
                                    nc.tensor.matmul(
                                        ps_pv[0][:, dcol:512],
                                        V[:, g, kc, :],
                                        pt[:, 0, dcol:512],
                                        start=(kc == kc0), stop=(kc == kcL),
                                        skip_group_check=True)
                                    nc.tensor.matmul(
                                        ps_pv[1][:, dcol:512],
                                        V[:, g, kc, :],
                                        pt[:, 1, dcol:512],
                                        start=(kc == kc0), stop=(kc == kcL),
                                        skip_group_check=True)
                                    # den rows on partitions 0 / 32 so one
                                    # reciprocal covers both heads
                                    nc.tensor.matmul(
                                        ps_dn[0:1, dcol:512],
                                        ones_c,
                                        pt[:, 0, dcol:512],
                                        start=(kc == kc0), stop=(kc == kcL),
                                        skip_group_check=True)
                                    nc.tensor.matmul(
                                        ps_dn[32:33, dcol:512],
                                        ones_c,
                                        pt[:, 1, dcol:512],
                                        start=(kc == kc0), stop=(kc == kcL),
                                        skip_group_check=True)

                                for kc, dcol, diag in act:
                                    st = psst.tile([128, 2, 512], f32,
                                                   tag="st")
                                    for i in range(2):
                                        nc.tensor.matmul(
                                            st[:, i, dcol:512],
                                            KT[:, g,
                                               kc * 128:(kc + 1) * 128],
                                            QT[:, h0 + i,
                                               qc * 512 + dcol:
                                               (qc + 1) * 512],
                                            start=True, stop=True)
                                    if prev is not None:
                                        pv_den(prev)
                                    pt = ptp.tile([128, 2, 512], bf16,
                                                  tag="pt")
                                    nc.scalar.activation(
                                        pt[:, :, dcol:512],
                                        st[:, :, dcol:512],
                                        AF.Exp, scale=scl)
                                    if diag:
                                        nc.vector.tensor_mul(
                                            pt[:, :, dcol:dcol + 128],
                                            pt[:, :, dcol:dcol + 128], tri2)
                                    prev = (kc, dcol, pt)
                                pv_den(prev)

                                # tail: one reciprocal for both heads
                                rec = recp.tile([33, 512], f32, tag="rec")
                                nc.vector.reciprocal(rec, ps_dn[0:33, :])
                                rec1 = recp.tile([1, 512], f32, tag="rec1")
                                nc.sync.dma_start(rec1, rec[32:33, :])
                                for i in range(2):
                                    un = unp.tile([128, 512], f32, tag="un")
                                    nc.vector.tensor_copy(un, ps_pv[i])
                                    r128 = unp.tile([128, 512], f32,
                                                    tag="r128")
                                    nc.gpsimd.partition_broadcast(
                                        r128, rec[0:1, :] if i == 0 else rec1)
                                    nc.vector.tensor_mul(
                                        attnT[:, h0 + i,
                                              qc * 512:(qc + 1) * 512],
                                        un, r128)

                # ---- output projection ----
                with (
                    tc.tile_pool(name="otp", bufs=2) as otp,
                    tc.tile_pool(name="pso", bufs=2, space="PSUM") as pso,
                ):
                    for qs in range(QS):
                        ps = pso.tile([128, D], f32)
                        for h in range(NH):
                            for ncH in range(4):
                                nc.tensor.matmul(
                                    ps[:, ncH * 512:(ncH + 1) * 512],
                                    attnT[:, h, qs * 128:(qs + 1) * 128],
                                    wot[:, h, ncH * 512:(ncH + 1) * 512],
                                    start=(h == 0), stop=(h == NH - 1),
                                    skip_group_check=True)
                        ot = otp.tile([128, D], f32, tag="ot")
                        nc.scalar.copy(ot, ps)
                        nc.sync.dma_start(
                            out_d[qs * 128:(qs + 1) * 128, :], ot)

    nc.compile()
    return nc


def _classify_mask(mask):
    """Per-[512q x 128k] tile -> active[qc] = [(kc, dcol, diag)].

    Verifies the mask is the causal+past pattern the kernel assumes:
    full tiles, skip tiles, and diagonal tiles of the form
    [masked rows | triangular block | allowed rows] split at dcol.
    """
    m = np.asarray(mask)
    tril = np.tril(np.ones((128, 128), bool))  # [q, k]: allow k <= q
    active = {}
    for qc in range(QC):
        lst = []
        for kc in range(KC):
            t = m[qc * 512:(qc + 1) * 512, kc * 128:(kc + 1) * 128]  # [q, k]
            if t.all():
                lst.append((kc, 0, False))
            elif not t.any():
                continue
            else:
                rows_any = np.nonzero(t.any(axis=1))[0]
                dcol = int(rows_any[0])
                assert dcol % 128 == 0, f"unexpected mask tile ({qc},{kc})"
                assert (t[dcol:dcol + 128] == tril).all(), \
                    f"non-causal tile ({qc},{kc})"
                assert t[dcol + 128:].all() or dcol + 128 >= 512
                assert not t[:dcol].any()
                lst.append((kc, dcol, True))
        assert lst and lst[0][1] == 0 and not lst[0][2], "first tile not full"
        active[qc] = lst
    return active


def _prep_in_maps(inputs):
    import ml_dtypes
    c32 = lambda a: np.ascontiguousarray(a, dtype=np.float32)
    c16 = lambda a: np.ascontiguousarray(a, dtype=ml_dtypes.bfloat16)
    x = np.asarray(inputs["x"], np.float32)
    rot = np.asarray(inputs["rotary_freqs"], np.float32)
    pk = np.asarray(inputs["past_k"], np.float32)
    pv = np.asarray(inputs["past_v"], np.float32)
    Wq = np.asarray(inputs["Wq"], np.float32)
    bq = np.asarray(inputs["bq"], np.float32)
    Wk = np.asarray(inputs["Wk"], np.float32)
    bk = np.asarray(inputs["bk"], np.float32)
    Wv = np.asarray(inputs["Wv"], np.float32)
    bv = np.asarray(inputs["bv"], np.float32)
    Wo = np.asarray(inputs["Wo"], np.float32)

    tri = np.triu(np.ones((128, 128), np.float32))  # [k, q]: allow k <= q
    tri2 = np.concatenate([tri, tri], axis=1)

    def tilize(w):
        # [K, M] -> [128, (K//128) * M], partition-contiguous runs
        K, M = w.shape
        return np.ascontiguousarray(
            w.reshape(K // 128, 128, M).transpose(1, 0, 2).reshape(128, -1))

    in_maps = []
    for c in range(NCORES):
        b, half = c // 2, c % 2
        h0 = half * NH
        g0 = half * NG
        qcols = np.concatenate(
            [Wq[:, (h0 + h) * HD + _PERM] for h in range(NH)], axis=1)
        kcols = np.concatenate(
            [Wk[:, (g0 + g) * HD + _PERM] for g in range(NG)], axis=1)
        bqk = np.stack(
            [bq[(h0 + h) * HD + _PERM] for h in range(NH)]
            + [bk[(g0 + g) * HD + _PERM] for g in range(NG)], axis=1)
        pkt = np.stack([pk[b, g0 + g][:, _PERM].T for g in range(NG)],
                       axis=1)                       # [128, NG, PAST]
        pvt = pv[b, g0:g0 + NG].reshape(NG, PAST // 128, 128, HD) \
            .transpose(2, 0, 1, 3)                   # [128, NG, kc, HD]
        wo = Wo[h0 * HD:(h0 + NH) * HD, :].reshape(NH, HD, D) \
            .transpose(1, 0, 2)                      # [128, NH, D]
        m = {
            "xt": c32(tilize(x[b].T)),
            "wqa": c32(tilize(qcols[:, 0:512])),
            "wqb": c32(tilize(qcols[:, 512:1024])),
            "wk": c32(tilize(kcols)),
            "wv": c32(tilize(Wv[:, g0 * HD:(g0 + NG) * HD])),
            "bqk": c32(bqk),
            "bv": c32(bv[g0 * HD:(g0 + NG) * HD][None, :]),
            "pkt": c16(pkt.reshape(128, -1)),
            "pv": c16(pvt.reshape(128, -1)),
            "rott": c32(rot.T),
            "wo": c16(wo.reshape(128, -1)),
            "tri": c16(tri2),
        }
        in_maps.append(m)
    return in_maps


def _run(inputs, trace=False):
    from concourse import bass_utils

    active = _classify_mask(inputs["mask"])
    key = tuple(sorted((qc, tuple(v)) for qc, v in active.items()))
    if key not in _PROG_CACHE:
        _PROG_CACHE[key] = _build_program(active)
    nc = _PROG_CACHE[key]

    in_maps = _prep_in_maps(inputs)
    res = bass_utils.run_bass_kernel_spmd(
        nc, in_maps, list(range(NCORES)), trace=trace,
        trace_cores=list(range(NCORES)) if trace else None)

    bo = np.asarray(inputs["bo"], np.float32)
    out = np.empty((B, LQ, D), np.float32)
    for b in range(B):
        out[b] = res.results[2 * b]["out"] + res.results[2 * b + 1]["out"] \
            + bo[None, :]
    return out, res


def kernel(**inputs) -> np.ndarray:
    out, _ = _run(inputs, trace=False)
    return out


# revision 9
# speedup vs baseline: 1.3826x; 1.3619x over previous
"""Cached grouped-query multi-head attention on 8 Trainium2 cores.

Sharding: core c -> batch b = c//2, head-half = c%2 (8 of 16 heads, 2 of 4
KV groups per core). Wq/Wk column-parallel, Wo row-parallel; the two
partial Wo products per batch are summed on the host (the "all-reduce"),
which also adds bo.

Host pre-layout (pure relayout, same class as the baseline's weight
permutations): x is shipped pre-transposed as xt[ki, ko, q], past_k
pre-transposed into KT layout, past_v in V-tile layout, rot transposed,
and every weight in a partition-contiguous tile layout, so the device
does ZERO PE transposes and every DMA is large-run.

Device kernel (per core, bf16 attention / f32r projections):
  K proj (interleaved with the chunked xt DMA) -> rope -> KT
  V proj -> V tiles; Q proj -> rope -> QT
  attention per (g, qc, head-pair): scores for 2 heads into a [128,2,512]
  2-bank psum supertile -> ONE exp for both heads (scale folded) ->
  diagonal tiles get shrunk matmuls plus a [128,128] triangular mask ->
  PV + den matmuls (den rows land on psum partitions 0/32 so one
  reciprocal covers both heads) -> normalize into attnT bf16
  Wo: full Wo prefetched bf16 during attention; attnT-stationary loop,
  psum [128,2048] accumulated over heads, scalar-copy + store per 128 q.
"""

import math
import sys

import numpy as np

sys.path.insert(0, "/opt/trn_rl_repo")

B, LQ, D = 4, 1024, 2048
H, G = 16, 4
HD = 128            # head dim
GS = H // G         # heads per group
PAST = 1024
LK = PAST + LQ      # 2048
NCORES = 8
NH = 8              # local heads per core
NG = 2              # local groups per core
KSUB = D // 128     # 16 contraction subtiles over D
QC = LQ // 512      # 2 query chunks of 512
QS = LQ // 128      # 8 query subtiles of 128
KC = LK // 128      # 16 key chunks of 128

_PERM = np.concatenate([np.arange(0, HD, 2), np.arange(1, HD, 2)])
_PROG_CACHE = {}


def _build_program(active):
    """active[qc] = [(kc, dcol, diag)]: dcol = first allowed query column
    (0 for full tiles); diag tiles have a triangular [128,128] block at
    query columns [dcol, dcol+128) and are fully allowed after it."""
    import concourse.bacc as bacc
    import concourse.mybir as mybir
    import concourse.tile as tile

    f32 = mybir.dt.float32
    f32r = mybir.dt.float32r
    bf16 = mybir.dt.bfloat16
    AF = mybir.ActivationFunctionType
    OP = mybir.AluOpType

    nc = bacc.Bacc("TRN2", target_bir_lowering=False, debug=False,
                   num_devices=NCORES)

    xt_d = nc.dram_tensor("xt", [128, KSUB * LQ], bf16,
                          kind="ExternalInput").ap()
    wqa_d = nc.dram_tensor("wqa", [128, KSUB * 512], bf16,
                           kind="ExternalInput").ap()
    wqb_d = nc.dram_tensor("wqb", [128, KSUB * 512], bf16,
                           kind="ExternalInput").ap()
    wk_d = nc.dram_tensor("wk", [128, KSUB * 256], bf16,
                          kind="ExternalInput").ap()
    wv_d = nc.dram_tensor("wv", [128, KSUB * 256], bf16,
                          kind="ExternalInput").ap()
    bqk_d = nc.dram_tensor("bqk", [128, NH + NG], f32,
                           kind="ExternalInput").ap()
    bv_d = nc.dram_tensor("bv", [1, NG * HD], f32, kind="ExternalInput").ap()
    pkt_d = nc.dram_tensor("pkt", [128, NG * PAST], bf16,
                           kind="ExternalInput").ap()
    pv_d = nc.dram_tensor("pv", [128, NG * PAST], bf16,
                          kind="ExternalInput").ap()
    rott_d = nc.dram_tensor("rott", [64, LQ], f32, kind="ExternalInput").ap()
    wo_d = nc.dram_tensor("wo", [128, NH * D], bf16,
                          kind="ExternalInput").ap()
    tri_d = nc.dram_tensor("tri", [128, 256], bf16, kind="ExternalInput").ap()
    out_d = nc.dram_tensor("out", [LQ, D], f32, kind="ExternalOutput").ap()

    scl = 1.0 / math.sqrt(HD)

    with tile.TileContext(nc) as tc:
        with (
            tc.tile_pool(name="const", bufs=1) as const,
            tc.tile_pool(name="persist", bufs=1) as persist,
        ):
            QT = persist.tile([128, NH, LQ], bf16)      # roped Q^T (perm rows)
            KT = persist.tile([128, NG, LK], bf16)      # K^T cache (perm rows)
            V = persist.tile([128, NG, KC, HD], bf16)   # [k, g, kc, hd]
            attnT = persist.tile([128, NH, LQ], bf16)   # normalized attn^T

            # critical-path loads first on the sync HWDGE queue
            with (
                tc.tile_pool(name="xtp", bufs=1) as xtp,
                tc.tile_pool(name="ropec", bufs=1) as ropec,
                tc.tile_pool(name="ropew", bufs=2) as ropew,
            ):
                xt = xtp.tile([128, KSUB, LQ], bf16)
                cosF = ropec.tile([128, LQ], f32)
                ssgnF = ropec.tile([128, LQ], f32)

                wkv_cm = tc.tile_pool(name="wkv", bufs=1)
                wkvp = wkv_cm.__enter__()
                wkt = wkvp.tile([128, KSUB, NG * HD], bf16, name="wk")
                wvt = wkvp.tile([128, KSUB, NG * HD], bf16, name="wv")

                # sync HWDGE queue: critical-path loads in consumption order
                nc.sync.dma_start(wkt, wk_d.rearrange("p (ko m) -> p ko m",
                                                      m=NG * HD))
                xt_r = xt_d.rearrange("p (ko q) -> p ko q", q=LQ)
                for j in range(4):
                    nc.sync.dma_start(xt[:, 4 * j:4 * (j + 1), :],
                                      xt_r[:, 4 * j:4 * (j + 1), :])
                nc.sync.dma_start(wvt, wv_d.rearrange("p (ko m) -> p ko m",
                                                      m=NG * HD))

                # secondary loads on the scalar HWDGE queue
                ones_f = const.tile([128, 1], f32)
                nc.gpsimd.memset(ones_f, 1.0)
                ones_c = const.tile([128, 1], bf16)
                nc.vector.tensor_copy(ones_c, ones_f)
                tri2 = const.tile([128, 2, 128], bf16)
                nc.scalar.dma_start(tri2,
                                    tri_d.rearrange("p (i f) -> p i f", f=128))
                bias_qk = const.tile([128, NH + NG], f32)
                nc.scalar.dma_start(bias_qk, bqk_d)
                bv_sb = const.tile([1, NG * HD], f32)
                nc.scalar.dma_start(bv_sb, bv_d)
                bv_rep = const.tile([128, NG * HD], f32)
                nc.gpsimd.partition_broadcast(bv_rep, bv_sb)
                nc.scalar.dma_start(
                    KT[:, :, 0:PAST],
                    pkt_d.rearrange("p (g f) -> p g f", g=NG))
                nc.scalar.dma_start(
                    V[:, :, 0:PAST // 128, :],
                    pv_d.rearrange("p (g kc hd) -> p g kc hd", g=NG, hd=HD))

                # rotary tables: rows 0:64 = even dims, 64:128 = odd;
                # ssgnF = -sin on top, +sin on bottom, so
                # roped = src*cosF + swap(src)*ssgnF
                rot_cm = tc.tile_pool(name="rotw", bufs=1)
                rotw = rot_cm.__enter__()
                rstage = rotw.tile([64, LQ], f32, name="rstage")
                s2 = rotw.tile([64, LQ], f32, name="s2")
                nc.scalar.dma_start(rstage, rott_d)
                negpi = const.tile([64, 1], f32)
                nc.gpsimd.memset(negpi, -math.pi)
                # -sin(x) = sin(x - pi); cos(x) = 1 - 2*sin^2(x/2)
                nc.scalar.activation(ssgnF[0:64], rstage, AF.Sin,
                                     bias=negpi)
                nc.scalar.activation(s2, rstage, AF.Sin, scale=0.5)
                nc.vector.tensor_mul(s2, s2, s2)
                nc.vector.tensor_scalar(cosF[0:64], s2, -2.0, 1.0,
                                        OP.mult, OP.add)
                nc.vector.tensor_scalar_mul(s2, ssgnF[0:64], -1.0)
                nc.sync.dma_start(ssgnF[64:128], s2)
                nc.sync.dma_start(cosF[64:128], cosF[0:64])
                rot_cm.__exit__(None, None, None)

                def rope(src, dst):
                    # src [128, LQ] f32 (clobbered); dst any dtype
                    swp = ropew.tile([128, LQ], f32, tag="swp")
                    nc.sync.dma_start(swp[0:64], src[64:128])
                    nc.sync.dma_start(swp[64:128], src[0:64])
                    t = ropew.tile([128, LQ], f32, tag="ropet")
                    nc.vector.tensor_mul(t, swp, ssgnF)
                    nc.vector.tensor_mul(src, src, cosF)
                    nc.vector.tensor_tensor(dst, src, t, OP.add)

                # ---- K proj, interleaved with the xt DMA chunks ----
                with (
                    tc.tile_pool(name="rawk", bufs=2) as rawk,
                    tc.tile_pool(name="pskp", bufs=1,
                                 space="PSUM") as pskp,
                ):
                    kps = [pskp.tile([128, 512], f32, name=f"kps{i}")
                           for i in range(4)]
                    for j in range(4):
                        for g in range(NG):
                            for qc in range(QC):
                                ps = kps[g * QC + qc]
                                for kk in range(4):
                                    ko = 4 * j + kk
                                    nc.tensor.matmul(
                                        ps,
                                        wkt[:, ko,
                                            g * HD:(g + 1) * HD],
                                        xt[:, ko,
                                           qc * 512:(qc + 1) * 512],
                                        start=(ko == 0),
                                        stop=(ko == KSUB - 1),
                                        skip_group_check=True)
                    kraws = []
                    for g in range(NG):
                        kraw = rawk.tile([128, LQ], f32, tag="kraw",
                                         name=f"kraw{g}")
                        for qc in range(QC):
                            nc.vector.tensor_scalar_add(
                                kraw[:, qc * 512:(qc + 1) * 512],
                                kps[g * QC + qc],
                                bias_qk[:, NH + g:NH + g + 1])
                        kraws.append(kraw)
                    for g in range(NG):
                        rope(kraws[g], KT[:, g, PAST:])

                # ---- V proj ----
                with tc.tile_pool(name="psv", bufs=4,
                                  space="PSUM") as psv:
                    for qs in range(QS):
                        ps = psv.tile([128, NG * HD], f32)
                        for ko in range(KSUB):
                            nc.tensor.matmul(
                                ps,
                                xt[:, ko, qs * 128:(qs + 1) * 128],
                                wvt[:, ko, :],
                                start=(ko == 0),
                                stop=(ko == KSUB - 1))
                        for g in range(NG):
                            nc.vector.tensor_tensor(
                                V[:, g, PAST // 128 + qs, :],
                                ps[:, g * HD:(g + 1) * HD],
                                bv_rep[:, g * HD:(g + 1) * HD],
                                OP.add)

                wkv_cm.__exit__(None, None, None)

                # ---- Q proj (both weight halves prefetched) ----
                wq_cm = tc.tile_pool(name="wqp", bufs=1)
                wqp = wq_cm.__enter__()
                wqt = [wqp.tile([128, KSUB, 512], bf16, name=f"wq{i}")
                       for i in range(2)]
                nc.sync.dma_start(
                    wqt[0], wqa_d.rearrange("p (ko m) -> p ko m", m=512))
                nc.sync.dma_start(
                    wqt[1], wqb_d.rearrange("p (ko m) -> p ko m", m=512))

                def q_proj(hh):
                    with (
                        tc.tile_pool(name="rawq", bufs=2) as rawq,
                        tc.tile_pool(name="psq", bufs=4, space="PSUM") as psq,
                    ):
                        for hl in range(4):
                            h = hh * 4 + hl
                            qraw = rawq.tile([128, LQ], f32, tag="qraw")
                            for qc in range(QC):
                                ps = psq.tile([128, 512], f32)
                                for ko in range(KSUB):
                                    nc.tensor.matmul(
                                        ps,
                                        wqt[hh][:, ko,
                                                hl * 128:(hl + 1) * 128],
                                        xt[:, ko, qc * 512:(qc + 1) * 512],
                                        start=(ko == 0),
                                        stop=(ko == KSUB - 1))
                                nc.vector.tensor_scalar_add(
                                    qraw[:, qc * 512:(qc + 1) * 512],
                                    ps, bias_qk[:, h:h + 1])
                            rope(qraw, QT[:, h, :])

                # ---- attention (g0 between the two Q-proj halves) ----
                with (
                    tc.tile_pool(name="wop", bufs=1) as wop,
                    tc.tile_pool(name="ptp", bufs=3) as ptp,
                    tc.tile_pool(name="unp", bufs=4) as unp,
                    tc.tile_pool(name="recp", bufs=1) as recp,
                ):
                    # prefetch full Wo (bf16) for phase 5
                    wot = wop.tile([128, NH, D], bf16)
                    nc.sync.dma_start(
                        wot, wo_d.rearrange("p (h n) -> p h n", n=D))

                    def attn_group(g):
                        with (
                            tc.tile_pool(name="psst", bufs=2,
                                         space="PSUM") as psst,
                            tc.tile_pool(name="pspv", bufs=2,
                                         space="PSUM") as pspv,
                            tc.tile_pool(name="psdn", bufs=2,
                                         space="PSUM") as psdn,
                        ):
                            for qc in range(QC):
                                act = active[qc]
                                kc0, kcL = act[0][0], act[-1][0]
                                for half in range(2):
                                    h0 = g * GS + half * 2
                                    ps_pv = [
                                        pspv.tile([128, 512], f32, tag="pv",
                                                  name=f"pv{half}{hi}")
                                        for hi in range(2)]
                                    ps_dn = psdn.tile([128, 512], f32,
                                                      tag="dn")
                                    nc.vector.memset(ps_dn, 1.0)
                                    prev = None

                                    def pv_den(item):
                                        kc, dcol, pt = item
                                        for i in range(2):
                                            nc.tensor.matmul(
                                                ps_pv[i][:, dcol:512],
                                                V[:, g, kc, :],
                                                pt[:, i, dcol:512],
                                                start=(kc == kc0),
                                                stop=(kc == kcL),
                                                skip_group_check=True)
                                        # den rows on psum partitions 0/32 so
                                        # one reciprocal covers both heads
                                        for i in range(2):
                                            nc.tensor.matmul(
                                                ps_dn[32 * i:32 * i + 1,
                                                      dcol:512],
                                                ones_c,
                                                pt[:, i, dcol:512],
                                                start=(kc == kc0),
                                                stop=(kc == kcL),
                                                skip_group_check=True)

                                    for kc, dcol, diag in act:
                                        st = psst.tile([128, 2, 512], f32,
                                                       tag="st")
                                        for i in range(2):
                                            nc.tensor.matmul(
                                                st[:, i, dcol:512],
                                                KT[:, g,
                                                   kc * 128:(kc + 1) * 128],
                                                QT[:, h0 + i,
                                                   qc * 512 + dcol:
                                                   (qc + 1) * 512],
                                                start=True, stop=True)
                                        if prev is not None:
                                            pv_den(prev)
                                        pt = ptp.tile([128, 2, 512], bf16,
                                                      tag="pt")
                                        nc.scalar.activation(
                                            pt[:, :, dcol:512],
                                            st[:, :, dcol:512],
                                            AF.Exp, scale=scl)
                                        if diag:
                                            for i in range(2):
                                                nc.vector.tensor_mul(
                                                    pt[:, i,
                                                       dcol:dcol + 128],
                                                    pt[:, i,
                                                       dcol:dcol + 128],
                                                    tri2[:, i, :])
                                        prev = (kc, dcol, pt)
                                    pv_den(prev)

                                    # tail: free the PV banks first, then one
                                    # reciprocal for both heads
                                    uns = []
                                    for i in range(2):
                                        un = unp.tile([128, 512], f32,
                                                      tag="un")
                                        nc.vector.tensor_copy(un, ps_pv[i])
                                        uns.append(un)
                                    rec = recp.tile([33, 512], f32, tag="rec")
                                    nc.vector.reciprocal(rec, ps_dn[0:33, :])
                                    rec1 = recp.tile([1, 512], f32,
                                                     tag="rec1")
                                    nc.sync.dma_start(rec1, rec[32:33, :])
                                    for i in range(2):
                                        r128 = unp.tile([128, 512], f32,
                                                        tag="r128")
                                        nc.gpsimd.partition_broadcast(
                                            r128,
                                            rec[0:1, :] if i == 0 else rec1)
                                        nc.vector.tensor_mul(
                                            attnT[:, h0 + i,
                                                  qc * 512:(qc + 1) * 512],
                                            uns[i], r128)

                    q_proj(0)
                    attn_group(0)
                    q_proj(1)
                    attn_group(1)

                    # ---- output projection ----
                    with tc.tile_pool(name="pso", bufs=2,
                                      space="PSUM") as pso:
                        for qs in range(QS):
                            ps = pso.tile([128, D], f32)
                            for h in range(NH):
                                for ncH in range(4):
                                    nc.tensor.matmul(
                                        ps[:, ncH * 512:(ncH + 1) * 512],
                                        attnT[:, h, qs * 128:(qs + 1) * 128],
                                        wot[:, h, ncH * 512:(ncH + 1) * 512],
                                        start=(h == 0), stop=(h == NH - 1),
                                        skip_group_check=True)
                            for ncH in range(4):
                                ot = unp.tile([128, 512], f32, tag="un")
                                nc.scalar.copy(
                                    ot, ps[:, ncH * 512:(ncH + 1) * 512])
                                nc.sync.dma_start(
                                    out_d[qs * 128:(qs + 1) * 128,
                                          ncH * 512:(ncH + 1) * 512], ot)

                wq_cm.__exit__(None, None, None)

    nc.compile()
    return nc


def _classify_mask(mask):
    """Per-[512q x 128k] tile -> active[qc] = [(kc, dcol, diag)].

    Verifies the mask is the causal+past pattern the kernel assumes:
    full tiles, skip tiles, and diagonal tiles of the form
    [masked rows | triangular block | allowed rows] split at dcol.
    """
    m = np.asarray(mask)
    tril = np.tril(np.ones((128, 128), bool))  # [q, k]: allow k <= q
    active = {}
    for qc in range(QC):
        lst = []
        for kc in range(KC):
            t = m[qc * 512:(qc + 1) * 512, kc * 128:(kc + 1) * 128]  # [q, k]
            if t.all():
                lst.append((kc, 0, False))
            elif not t.any():
                continue
            else:
                rows_any = np.nonzero(t.any(axis=1))[0]
                dcol = int(rows_any[0])
                assert dcol % 128 == 0, f"unexpected mask tile ({qc},{kc})"
                assert (t[dcol:dcol + 128] == tril).all(), \
                    f"non-causal tile ({qc},{kc})"
                assert t[dcol + 128:].all() or dcol + 128 >= 512
                assert not t[:dcol].any()
                lst.append((kc, dcol, True))
        assert lst and lst[0][1] == 0 and not lst[0][2], "first tile not full"
        active[qc] = lst
    return active


def _prep_in_maps(inputs):
    import ml_dtypes
    c32 = lambda a: np.ascontiguousarray(a, dtype=np.float32)
    c16 = lambda a: np.ascontiguousarray(a, dtype=ml_dtypes.bfloat16)
    x = np.asarray(inputs["x"], np.float32)
    rot = np.asarray(inputs["rotary_freqs"], np.float32)
    pk = np.asarray(inputs["past_k"], np.float32)
    pv = np.asarray(inputs["past_v"], np.float32)
    Wq = np.asarray(inputs["Wq"], np.float32)
    bq = np.asarray(inputs["bq"], np.float32)
    Wk = np.asarray(inputs["Wk"], np.float32)
    bk = np.asarray(inputs["bk"], np.float32)
    Wv = np.asarray(inputs["Wv"], np.float32)
    bv = np.asarray(inputs["bv"], np.float32)
    Wo = np.asarray(inputs["Wo"], np.float32)

    tri = np.triu(np.ones((128, 128), np.float32))  # [k, q]: allow k <= q
    tri2 = np.concatenate([tri, tri], axis=1)

    def tilize(w):
        # [K, M] -> [128, (K//128) * M], partition-contiguous runs
        K, M = w.shape
        return np.ascontiguousarray(
            w.reshape(K // 128, 128, M).transpose(1, 0, 2).reshape(128, -1))

    in_maps = []
    for c in range(NCORES):
        b, half = c // 2, c % 2
        h0 = half * NH
        g0 = half * NG
        qcols = np.concatenate(
            [Wq[:, (h0 + h) * HD + _PERM] for h in range(NH)], axis=1)
        kcols = np.concatenate(
            [Wk[:, (g0 + g) * HD + _PERM] for g in range(NG)], axis=1)
        bqk = np.stack(
            [bq[(h0 + h) * HD + _PERM] for h in range(NH)]
            + [bk[(g0 + g) * HD + _PERM] for g in range(NG)], axis=1)
        pkt = np.stack([pk[b, g0 + g][:, _PERM].T for g in range(NG)],
                       axis=1)                       # [128, NG, PAST]
        pvt = pv[b, g0:g0 + NG].reshape(NG, PAST // 128, 128, HD) \
            .transpose(2, 0, 1, 3)                   # [128, NG, kc, HD]
        wo = Wo[h0 * HD:(h0 + NH) * HD, :].reshape(NH, HD, D) \
            .transpose(1, 0, 2)                      # [128, NH, D]
        m = {
            "xt": c16(tilize(x[b].T)),
            "wqa": c16(tilize(qcols[:, 0:512])),
            "wqb": c16(tilize(qcols[:, 512:1024])),
            "wk": c16(tilize(kcols)),
            "wv": c16(tilize(Wv[:, g0 * HD:(g0 + NG) * HD])),
            "bqk": c32(bqk),
            "bv": c32(bv[g0 * HD:(g0 + NG) * HD][None, :]),
            "pkt": c16(pkt.reshape(128, -1)),
            "pv": c16(pvt.reshape(128, -1)),
            "rott": c32(rot.T),
            "wo": c16(wo.reshape(128, -1)),
            "tri": c16(tri2),
        }
        in_maps.append(m)
    return in_maps


def _run(inputs, trace=False):
    from concourse import bass_utils

    active = _classify_mask(inputs["mask"])
    key = tuple(sorted((qc, tuple(v)) for qc, v in active.items()))
    if key not in _PROG_CACHE:
        _PROG_CACHE[key] = _build_program(active)
    nc = _PROG_CACHE[key]

    in_maps = _prep_in_maps(inputs)
    res = bass_utils.run_bass_kernel_spmd(
        nc, in_maps, list(range(NCORES)), trace=trace,
        trace_cores=list(range(NCORES)) if trace else None)

    bo = np.asarray(inputs["bo"], np.float32)
    out = np.empty((B, LQ, D), np.float32)
    for b in range(B):
        out[b] = res.results[2 * b]["out"] + res.results[2 * b + 1]["out"] \
            + bo[None, :]
    return out, res


def kernel(**inputs) -> np.ndarray:
    out, _ = _run(inputs, trace=False)
    return out


# revision 13
# speedup vs baseline: 1.4677x; 1.0615x over previous
"""Cached grouped-query multi-head attention on 8 Trainium2 cores.

Sharding: core c -> batch b = c//2, head-half = c%2 (8 of 16 heads, 2 of 4
KV groups per core). Wq/Wk column-parallel, Wo row-parallel; the two
partial Wo products per batch are summed on the host (the "all-reduce"),
which also adds bo.

Host pre-layout (pure relayout, same class as the baseline's weight
permutations): x is shipped pre-transposed as xt[ki, ko, q], past_k
pre-transposed into KT layout, past_v in V-tile layout, rot transposed,
and every weight in a partition-contiguous tile layout, so the device
does ZERO PE transposes and every DMA is large-run.

Device kernel (per core, bf16 attention / f32r projections):
  K proj (interleaved with the chunked xt DMA) -> rope -> KT
  V proj -> V tiles; Q proj -> rope -> QT
  attention per (g, qc, head-pair): scores for 2 heads into a [128,2,512]
  2-bank psum supertile -> ONE exp for both heads (scale folded) ->
  diagonal tiles get shrunk matmuls plus a [128,128] triangular mask ->
  PV + den matmuls (den rows land on psum partitions 0/32 so one
  reciprocal covers both heads) -> normalize into attnT bf16
  Wo: full Wo prefetched bf16 during attention; attnT-stationary loop,
  psum [128,2048] accumulated over heads, scalar-copy + store per 128 q.
"""

import math
import sys

import numpy as np

sys.path.insert(0, "/opt/trn_rl_repo")

B, LQ, D = 4, 1024, 2048
H, G = 16, 4
HD = 128            # head dim
GS = H // G         # heads per group
PAST = 1024
LK = PAST + LQ      # 2048
NCORES = 8
NH = 8              # local heads per core
NG = 2              # local groups per core
KSUB = D // 128     # 16 contraction subtiles over D
QC = LQ // 512      # 2 query chunks of 512
QS = LQ // 128      # 8 query subtiles of 128
KC = LK // 128      # 16 key chunks of 128

_PERM = np.concatenate([np.arange(0, HD, 2), np.arange(1, HD, 2)])
_PROG_CACHE = {}


def _build_program(active):
    """active[qc] = [(kc, dcol, diag)]: dcol = first allowed query column
    (0 for full tiles); diag tiles have a triangular [128,128] block at
    query columns [dcol, dcol+128) and are fully allowed after it."""
    import concourse.bacc as bacc
    import concourse.mybir as mybir
    import concourse.tile as tile

    f32 = mybir.dt.float32
    f32r = mybir.dt.float32r
    bf16 = mybir.dt.bfloat16
    AF = mybir.ActivationFunctionType
    OP = mybir.AluOpType

    nc = bacc.Bacc("TRN2", target_bir_lowering=False, debug=False,
                   num_devices=NCORES)

    xt_d = nc.dram_tensor("xt", [128, KSUB * LQ], bf16,
                          kind="ExternalInput").ap()
    wqa_d = nc.dram_tensor("wqa", [128, KSUB * 512], bf16,
                           kind="ExternalInput").ap()
    wqb_d = nc.dram_tensor("wqb", [128, KSUB * 512], bf16,
                           kind="ExternalInput").ap()
    wk_d = nc.dram_tensor("wk", [128, KSUB * 256], bf16,
                          kind="ExternalInput").ap()
    wv_d = nc.dram_tensor("wv", [128, KSUB * 256], bf16,
                          kind="ExternalInput").ap()
    bqk_d = nc.dram_tensor("bqk", [128, NH + NG], f32,
                           kind="ExternalInput").ap()
    bv_d = nc.dram_tensor("bv", [1, NG * HD], f32, kind="ExternalInput").ap()
    pkt_d = nc.dram_tensor("pkt", [128, NG * PAST], bf16,
                           kind="ExternalInput").ap()
    pv_d = nc.dram_tensor("pv", [128, NG * PAST], bf16,
                          kind="ExternalInput").ap()
    rott_d = nc.dram_tensor("rott", [64, LQ], f32, kind="ExternalInput").ap()
    wo_d = nc.dram_tensor("wo", [128, NH * D], bf16,
                          kind="ExternalInput").ap()
    tri_d = nc.dram_tensor("tri", [128, 256], bf16, kind="ExternalInput").ap()
    out_d = nc.dram_tensor("out", [LQ, D], f32, kind="ExternalOutput").ap()

    scl = 1.0 / math.sqrt(HD)

    with tile.TileContext(nc) as tc:
        with (
            tc.tile_pool(name="const", bufs=1) as const,
            tc.tile_pool(name="persist", bufs=1) as persist,
        ):
            QT = persist.tile([128, NH, LQ], bf16)      # roped Q^T (perm rows)
            KT = persist.tile([128, NG, LK], bf16)      # K^T cache (perm rows)
            V = persist.tile([128, NG, KC, HD], bf16)   # [k, g, kc, hd]
            attnT = persist.tile([128, NH, LQ], bf16)   # normalized attn^T

            # critical-path loads first on the sync HWDGE queue
            with (
                tc.tile_pool(name="xtp", bufs=1) as xtp,
                tc.tile_pool(name="ropec", bufs=1) as ropec,
                tc.tile_pool(name="ropew", bufs=2) as ropew,
            ):
                xt = xtp.tile([128, KSUB, LQ], bf16)
                cosF = ropec.tile([128, LQ], f32)
                ssgnF = ropec.tile([128, LQ], f32)

                wq_cm = tc.tile_pool(name="wqp", bufs=1)
                wqp = wq_cm.__enter__()
                wqt = [wqp.tile([128, KSUB, 512], bf16, name=f"wq{i}")
                       for i in range(2)]
                wkv_cm = tc.tile_pool(name="wkv", bufs=1)
                wkvp = wkv_cm.__enter__()
                wkt = wkvp.tile([128, KSUB, NG * HD], bf16, name="wk")
                wvt = wkvp.tile([128, KSUB, NG * HD], bf16, name="wv")

                # sync HWDGE queue: wk + xt in consumption order,
                # fine-grained so the first K matmuls start early
                wk_r = wk_d.rearrange("p (ko m) -> p ko m", m=NG * HD)
                xt_r = xt_d.rearrange("p (ko q) -> p ko q", q=LQ)
                for j in range(4):
                    nc.sync.dma_start(wkt[:, 4 * j:4 * (j + 1), :],
                                      wk_r[:, 4 * j:4 * (j + 1), :])
                    nc.sync.dma_start(xt[:, 4 * j:4 * (j + 1), :],
                                      xt_r[:, 4 * j:4 * (j + 1), :])

                # scalar HWDGE queue: rotary stage, V/Q weights, KV cache
                rot_cm = tc.tile_pool(name="rotw", bufs=1)
                rotw = rot_cm.__enter__()
                rstage = rotw.tile([64, LQ], f32, name="rstage")
                s2 = rotw.tile([64, LQ], f32, name="s2")
                nc.scalar.dma_start(rstage, rott_d)
                nc.scalar.dma_start(
                    wvt, wv_d.rearrange("p (ko m) -> p ko m", m=NG * HD))
                nc.scalar.dma_start(
                    wqt[0], wqa_d.rearrange("p (ko m) -> p ko m", m=512))
                nc.scalar.dma_start(
                    wqt[1], wqb_d.rearrange("p (ko m) -> p ko m", m=512))
                ones_f = const.tile([128, 1], f32)
                nc.gpsimd.memset(ones_f, 1.0)
                ones_c = const.tile([128, 1], bf16)
                nc.vector.tensor_copy(ones_c, ones_f)
                tri2 = const.tile([128, 2, 128], bf16)
                nc.scalar.dma_start(tri2,
                                    tri_d.rearrange("p (i f) -> p i f", f=128))
                bias_qk = const.tile([128, NH + NG], f32)
                nc.scalar.dma_start(bias_qk, bqk_d)
                bv_sb = const.tile([1, NG * HD], f32)
                nc.scalar.dma_start(bv_sb, bv_d)
                bv_rep = const.tile([128, NG * HD], f32)
                nc.gpsimd.partition_broadcast(bv_rep, bv_sb)
                nc.scalar.dma_start(
                    KT[:, :, 0:PAST],
                    pkt_d.rearrange("p (g f) -> p g f", g=NG))
                nc.scalar.dma_start(
                    V[:, :, 0:PAST // 128, :],
                    pv_d.rearrange("p (g kc hd) -> p g kc hd", g=NG, hd=HD))

                # rotary tables: rows 0:64 = even dims, 64:128 = odd;
                # ssgnF = -sin on top, +sin on bottom, so
                # roped = src*cosF + swap(src)*ssgnF
                negpi = const.tile([64, 1], f32)
                nc.gpsimd.memset(negpi, -math.pi)
                # -sin(x) = sin(x - pi); cos(x) = 1 - 2*sin^2(x/2)
                nc.scalar.activation(ssgnF[0:64], rstage, AF.Sin,
                                     bias=negpi)
                nc.scalar.activation(s2, rstage, AF.Sin, scale=0.5)
                nc.vector.tensor_mul(s2, s2, s2)
                nc.vector.tensor_scalar(cosF[0:64], s2, -2.0, 1.0,
                                        OP.mult, OP.add)
                nc.vector.tensor_scalar_mul(s2, ssgnF[0:64], -1.0)
                nc.sync.dma_start(ssgnF[64:128], s2)
                nc.sync.dma_start(cosF[64:128], cosF[0:64])
                rot_cm.__exit__(None, None, None)

                def rope(src, dst):
                    # src [128, LQ] f32 (clobbered); dst any dtype
                    swp = ropew.tile([128, LQ], f32, tag="swp")
                    nc.sync.dma_start(swp[0:64], src[64:128])
                    nc.sync.dma_start(swp[64:128], src[0:64])
                    t = ropew.tile([128, LQ], f32, tag="ropet")
                    nc.vector.tensor_mul(t, swp, ssgnF)
                    nc.vector.tensor_mul(src, src, cosF)
                    nc.vector.tensor_tensor(dst, src, t, OP.add)

                # ---- K proj, interleaved with the xt DMA chunks ----
                with (
                    tc.tile_pool(name="rawk", bufs=2) as rawk,
                    tc.tile_pool(name="pskp", bufs=1,
                                 space="PSUM") as pskp,
                ):
                    kps = [pskp.tile([128, 512], f32, name=f"kps{i}")
                           for i in range(4)]
                    for j in range(4):
                        for g in range(NG):
                            for qc in range(QC):
                                ps = kps[g * QC + qc]
                                for kk in range(4):
                                    ko = 4 * j + kk
                                    nc.tensor.matmul(
                                        ps,
                                        wkt[:, ko,
                                            g * HD:(g + 1) * HD],
                                        xt[:, ko,
                                           qc * 512:(qc + 1) * 512],
                                        start=(ko == 0),
                                        stop=(ko == KSUB - 1),
                                        skip_group_check=True)
                    kraws = []
                    for g in range(NG):
                        kraw = rawk.tile([128, LQ], f32, tag="kraw",
                                         name=f"kraw{g}")
                        for qc in range(QC):
                            nc.vector.tensor_scalar_add(
                                kraw[:, qc * 512:(qc + 1) * 512],
                                kps[g * QC + qc],
                                bias_qk[:, NH + g:NH + g + 1])
                        kraws.append(kraw)
                    for g in range(NG):
                        rope(kraws[g], KT[:, g, PAST:])

                # ---- V proj ----
                with tc.tile_pool(name="psv", bufs=4,
                                  space="PSUM") as psv:
                    for qs in range(QS):
                        ps = psv.tile([128, NG * HD], f32)
                        for ko in range(KSUB):
                            nc.tensor.matmul(
                                ps,
                                xt[:, ko, qs * 128:(qs + 1) * 128],
                                wvt[:, ko, :],
                                start=(ko == 0),
                                stop=(ko == KSUB - 1))
                        for g in range(NG):
                            nc.vector.tensor_tensor(
                                V[:, g, PAST // 128 + qs, :],
                                ps[:, g * HD:(g + 1) * HD],
                                bv_rep[:, g * HD:(g + 1) * HD],
                                OP.add)

                wkv_cm.__exit__(None, None, None)

                # ---- Q proj (both weight halves prefetched) ----
                def q_proj(hh):
                    with (
                        tc.tile_pool(name="rawq", bufs=2) as rawq,
                        tc.tile_pool(name="psq", bufs=4, space="PSUM") as psq,
                    ):
                        for hl in range(4):
                            h = hh * 4 + hl
                            qraw = rawq.tile([128, LQ], f32, tag="qraw")
                            for qc in range(QC):
                                ps = psq.tile([128, 512], f32)
                                for ko in range(KSUB):
                                    nc.tensor.matmul(
                                        ps,
                                        wqt[hh][:, ko,
                                                hl * 128:(hl + 1) * 128],
                                        xt[:, ko, qc * 512:(qc + 1) * 512],
                                        start=(ko == 0),
                                        stop=(ko == KSUB - 1))
                                nc.vector.tensor_scalar_add(
                                    qraw[:, qc * 512:(qc + 1) * 512],
                                    ps, bias_qk[:, h:h + 1])
                            rope(qraw, QT[:, h, :])

                # ---- attention (g0 between the two Q-proj halves) ----
                with (
                    tc.tile_pool(name="wop", bufs=1) as wop,
                    tc.tile_pool(name="ptp", bufs=3) as ptp,
                    tc.tile_pool(name="unp", bufs=4) as unp,
                    tc.tile_pool(name="recp", bufs=1) as recp,
                ):
                    # prefetch full Wo (bf16) for phase 5
                    wot = wop.tile([128, NH, D], bf16)
                    nc.sync.dma_start(
                        wot, wo_d.rearrange("p (h n) -> p h n", n=D))

                    def attn_group(g):
                        with (
                            tc.tile_pool(name="psst", bufs=2,
                                         space="PSUM") as psst,
                            tc.tile_pool(name="pspv", bufs=2,
                                         space="PSUM") as pspv,
                            tc.tile_pool(name="psdn", bufs=2,
                                         space="PSUM") as psdn,
                        ):
                            for qc in range(QC):
                                act = active[qc]
                                kc0, kcL = act[0][0], act[-1][0]
                                for half in range(2):
                                    h0 = g * GS + half * 2
                                    ps_pv = [
                                        pspv.tile([128, 512], f32, tag="pv",
                                                  name=f"pv{half}{hi}")
                                        for hi in range(2)]
                                    ps_dn = psdn.tile([128, 512], f32,
                                                      tag="dn")
                                    nc.vector.memset(ps_dn, 1.0)
                                    pend = []

                                    def pv_den(item):
                                        kc, dcol, pt = item
                                        for i in range(2):
                                            nc.tensor.matmul(
                                                ps_pv[i][:, dcol:512],
                                                V[:, g, kc, :],
                                                pt[:, i, dcol:512],
                                                start=(kc == kc0),
                                                stop=(kc == kcL),
                                                skip_group_check=True)
                                        # den rows on psum partitions 0/32 so
                                        # one reciprocal covers both heads
                                        for i in range(2):
                                            nc.tensor.matmul(
                                                ps_dn[32 * i:32 * i + 1,
                                                      dcol:512],
                                                ones_c,
                                                pt[:, i, dcol:512],
                                                start=(kc == kc0),
                                                stop=(kc == kcL),
                                                skip_group_check=True)

                                    for kc, dcol, diag in act:
                                        st = psst.tile([128, 2, 512], f32,
                                                       tag="st")
                                        for i in range(2):
                                            nc.tensor.matmul(
                                                st[:, i, dcol:512],
                                                KT[:, g,
                                                   kc * 128:(kc + 1) * 128],
                                                QT[:, h0 + i,
                                                   qc * 512 + dcol:
                                                   (qc + 1) * 512],
                                                start=True, stop=True)
                                        if len(pend) == 2:
                                            pv_den(pend.pop(0))
                                        pt = ptp.tile([128, 2, 512], bf16,
                                                      tag="pt")
                                        nc.scalar.activation(
                                            pt[:, :, dcol:512],
                                            st[:, :, dcol:512],
                                            AF.Exp, scale=scl)
                                        if diag:
                                            for i in range(2):
                                                nc.vector.tensor_mul(
                                                    pt[:, i,
                                                       dcol:dcol + 128],
                                                    pt[:, i,
                                                       dcol:dcol + 128],
                                                    tri2[:, i, :])
                                        pend.append((kc, dcol, pt))
                                    for item in pend:
                                        pv_den(item)

                                    # tail: free the PV banks first, then one
                                    # reciprocal for both heads
                                    uns = []
                                    for i in range(2):
                                        un = unp.tile([128, 512], f32,
                                                      tag="un")
                                        nc.vector.tensor_copy(un, ps_pv[i])
                                        uns.append(un)
                                    rec = recp.tile([33, 512], f32, tag="rec")
                                    nc.vector.reciprocal(rec, ps_dn[0:33, :])
                                    rec1 = recp.tile([1, 512], f32,
                                                     tag="rec1")
                                    nc.sync.dma_start(rec1, rec[32:33, :])
                                    for i in range(2):
                                        r128 = unp.tile([128, 512], f32,
                                                        tag="r128")
                                        nc.gpsimd.partition_broadcast(
                                            r128,
                                            rec[0:1, :] if i == 0 else rec1)
                                        nc.vector.tensor_mul(
                                            attnT[:, h0 + i,
                                                  qc * 512:(qc + 1) * 512],
                                            uns[i], r128)

                    q_proj(0)
                    attn_group(0)
                    q_proj(1)
                    attn_group(1)

                    # ---- output projection ----
                    with tc.tile_pool(name="pso", bufs=2,
                                      space="PSUM") as pso:
                        for qs in range(QS):
                            ps = pso.tile([128, D], f32)
                            for h in range(NH):
                                for ncH in range(4):
                                    nc.tensor.matmul(
                                        ps[:, ncH * 512:(ncH + 1) * 512],
                                        attnT[:, h, qs * 128:(qs + 1) * 128],
                                        wot[:, h, ncH * 512:(ncH + 1) * 512],
                                        start=(h == 0), stop=(h == NH - 1),
                                        skip_group_check=True)
                            for ncH in range(4):
                                ot = unp.tile([128, 512], f32, tag="un")
                                nc.scalar.copy(
                                    ot, ps[:, ncH * 512:(ncH + 1) * 512])
                                nc.sync.dma_start(
                                    out_d[qs * 128:(qs + 1) * 128,
                                          ncH * 512:(ncH + 1) * 512], ot)

                wq_cm.__exit__(None, None, None)

    nc.compile()
    return nc


def _classify_mask(mask):
    """Per-[512q x 128k] tile -> active[qc] = [(kc, dcol, diag)].

    Verifies the mask is the causal+past pattern the kernel assumes:
    full tiles, skip tiles, and diagonal tiles of the form
    [masked rows | triangular block | allowed rows] split at dcol.
    """
    m = np.asarray(mask)
    tril = np.tril(np.ones((128, 128), bool))  # [q, k]: allow k <= q
    active = {}
    for qc in range(QC):
        lst = []
        for kc in range(KC):
            t = m[qc * 512:(qc + 1) * 512, kc * 128:(kc + 1) * 128]  # [q, k]
            if t.all():
                lst.append((kc, 0, False))
            elif not t.any():
                continue
            else:
                rows_any = np.nonzero(t.any(axis=1))[0]
                dcol = int(rows_any[0])
                assert dcol % 128 == 0, f"unexpected mask tile ({qc},{kc})"
                assert (t[dcol:dcol + 128] == tril).all(), \
                    f"non-causal tile ({qc},{kc})"
                assert t[dcol + 128:].all() or dcol + 128 >= 512
                assert not t[:dcol].any()
                lst.append((kc, dcol, True))
        assert lst and lst[0][1] == 0 and not lst[0][2], "first tile not full"
        active[qc] = lst
    return active


def _prep_in_maps(inputs):
    import ml_dtypes
    c32 = lambda a: np.ascontiguousarray(a, dtype=np.float32)
    c16 = lambda a: np.ascontiguousarray(a, dtype=ml_dtypes.bfloat16)
    x = np.asarray(inputs["x"], np.float32)
    rot = np.asarray(inputs["rotary_freqs"], np.float32)
    pk = np.asarray(inputs["past_k"], np.float32)
    pv = np.asarray(inputs["past_v"], np.float32)
    Wq = np.asarray(inputs["Wq"], np.float32)
    bq = np.asarray(inputs["bq"], np.float32)
    Wk = np.asarray(inputs["Wk"], np.float32)
    bk = np.asarray(inputs["bk"], np.float32)
    Wv = np.asarray(inputs["Wv"], np.float32)
    bv = np.asarray(inputs["bv"], np.float32)
    Wo = np.asarray(inputs["Wo"], np.float32)

    tri = np.triu(np.ones((128, 128), np.float32))  # [k, q]: allow k <= q
    tri2 = np.concatenate([tri, tri], axis=1)

    def tilize(w):
        # [K, M] -> [128, (K//128) * M], partition-contiguous runs
        K, M = w.shape
        return np.ascontiguousarray(
            w.reshape(K // 128, 128, M).transpose(1, 0, 2).reshape(128, -1))

    in_maps = []
    for c in range(NCORES):
        b, half = c // 2, c % 2
        h0 = half * NH
        g0 = half * NG
        qcols = np.concatenate(
            [Wq[:, (h0 + h) * HD + _PERM] for h in range(NH)], axis=1)
        kcols = np.concatenate(
            [Wk[:, (g0 + g) * HD + _PERM] for g in range(NG)], axis=1)
        bqk = np.stack(
            [bq[(h0 + h) * HD + _PERM] for h in range(NH)]
            + [bk[(g0 + g) * HD + _PERM] for g in range(NG)], axis=1)
        pkt = np.stack([pk[b, g0 + g][:, _PERM].T for g in range(NG)],
                       axis=1)                       # [128, NG, PAST]
        pvt = pv[b, g0:g0 + NG].reshape(NG, PAST // 128, 128, HD) \
            .transpose(2, 0, 1, 3)                   # [128, NG, kc, HD]
        wo = Wo[h0 * HD:(h0 + NH) * HD, :].reshape(NH, HD, D) \
            .transpose(1, 0, 2)                      # [128, NH, D]
        m = {
            "xt": c16(tilize(x[b].T)),
            "wqa": c16(tilize(qcols[:, 0:512])),
            "wqb": c16(tilize(qcols[:, 512:1024])),
            "wk": c16(tilize(kcols)),
            "wv": c16(tilize(Wv[:, g0 * HD:(g0 + NG) * HD])),
            "bqk": c32(bqk),
            "bv": c32(bv[g0 * HD:(g0 + NG) * HD][None, :]),
            "pkt": c16(pkt.reshape(128, -1)),
            "pv": c16(pvt.reshape(128, -1)),
            "rott": c32(rot.T),
            "wo": c16(wo.reshape(128, -1)),
            "tri": c16(tri2),
        }
        in_maps.append(m)
    return in_maps


def _run(inputs, trace=False):
    from concourse import bass_utils

    active = _classify_mask(inputs["mask"])
    key = tuple(sorted((qc, tuple(v)) for qc, v in active.items()))
    if key not in _PROG_CACHE:
        _PROG_CACHE[key] = _build_program(active)
    nc = _PROG_CACHE[key]

    in_maps = _prep_in_maps(inputs)
    res = bass_utils.run_bass_kernel_spmd(
        nc, in_maps, list(range(NCORES)), trace=trace,
        trace_cores=list(range(NCORES)) if trace else None)

    bo = np.asarray(inputs["bo"], np.float32)
    out = np.empty((B, LQ, D), np.float32)
    for b in range(B):
        out[b] = res.results[2 * b]["out"] + res.results[2 * b + 1]["out"] \
            + bo[None, :]
    return out, res


def kernel(**inputs) -> np.ndarray:
    out, _ = _run(inputs, trace=False)
    return out
